# revision 16
# baseline (speedup 1.0000x reference)
"""Causal self-attention (B=4, S=2048, E=1024, D=128, single head) on 8 TRN2 cores.

Sharding: core c = 2*b + h handles batch b; the two cores of a pair split the
causal key range by k-tile parity (h=0 even 128-row k-tiles, h=1 odd).  All 8
cores run the *same* instruction stream; per-core differences live in DRAM
data (host-permuted x columns and a parity-dependent mask tile).

Query storage order: position p <-> global q-tile g = p ^ h.  This makes the
causal structure suffix-contiguous and parity-independent: attention block b
covers storage positions 4b..4b+3 (= global tiles {4b..4b+3} for both
parities); its own-parity k-tiles are exactly i = 0..2b+1, where k-tile i is
"in-block" iff i >= 2b.  Out-of-block k-tiles (i < 2b) need no mask and a full
512-wide scores stream; the two in-block k-tiles (i = 2b, 2b+1) stream only
the causal suffix (cols 0/256..512) with a single 256-wide host mask segment
[tri | 0 or -inf] pre-loaded into PSUM via an identity matmul (start=False
accumulation keeps masking off the DVE/ACT chain).  vs. the parity-blocked
predecessor this removes all rank-1 code masks and ~25% of scores/pv/sums
matmul columns.

All matmuls run in bf16 (fp32 PSUM accumulation).  K/V projections cover the
own-parity half; Q covers all 2048 queries (kv-parity tiles land at even
storage p via a strided write, oth-parity at odd p).  pv/sum/proj PSUM pools
are double-buffered so copy-outs overlap the next block's matmuls.
Each core emits unnormalized PV partials (pvT [128 d, 2048 q]) and softmax
denominators (sums [1, 2048]); the host combines the pair:
  out[b] = ((pv0 + pv1) / (s0 + s1)).T  (+ per-core q-column de-permutation)
"""

import os

os.environ.setdefault("MYCRO_LOCAL_CACHE", "1")

import ml_dtypes
import numpy as np

B, S, E, D = 4, 2048, 1024, 128
P = 128
NT = S // P          # 16 global k-tiles per batch
LT = NT // 2         # 8 local (per-core) k-tiles
NQB = 4              # 512-wide query blocks
QBW = 512
NEB = E // P         # 8 e-tiles
SCALE = 1.0 / float(np.sqrt(D))
NEG = -1.0e30

TRACE = False        # set by test.py for profiling runs
TRACE_KW = {}

_CACHE = {}


def _build_module(reps=1):
    from contextlib import ExitStack

    import concourse.bacc as bacc
    import concourse.mybir as mybir
    import concourse.tile as tile

    f32 = mybir.dt.float32
    bf16 = mybir.dt.bfloat16

    nc = bacc.Bacc("TRN2", target_bir_lowering=False, debug=False, num_devices=8)

    xt_kv = nc.dram_tensor("xt_kv", [E, S // 2], bf16, kind="ExternalInput").ap()
    xt_oth = nc.dram_tensor("xt_oth", [E, S // 2], bf16, kind="ExternalInput").ap()
    wq_d = nc.dram_tensor("wq", [E, D], bf16, kind="ExternalInput").ap()
    wk_d = nc.dram_tensor("wk", [E, D], bf16, kind="ExternalInput").ap()
    wv_d = nc.dram_tensor("wv", [E, D], bf16, kind="ExternalInput").ap()
    bq_d = nc.dram_tensor("bq", [D], f32, kind="ExternalInput").ap()  # pre-scaled
    bk_d = nc.dram_tensor("bk", [D], f32, kind="ExternalInput").ap()
    bv_d = nc.dram_tensor("bv", [D], f32, kind="ExternalInput").ap()
    # [k=128, 2*128 q] = [tri | h==0 ? 0 : -inf]
    mask_d = nc.dram_tensor("mask2", [P, 2 * P], bf16, kind="ExternalInput").ap()
    identb_d = nc.dram_tensor("identb", [P, P], bf16, kind="ExternalInput").ap()
    onesc_d = nc.dram_tensor("onesc", [P, 1], bf16, kind="ExternalInput").ap()
    pvt_d = nc.dram_tensor("pvt", [D, S], f32, kind="ExternalOutput").ap()
    sums_d = nc.dram_tensor("sums", [1, S], f32, kind="ExternalOutput").ap()

    with tile.TileContext(nc) as tc, ExitStack() as ctx:
        singles = ctx.enter_context(tc.tile_pool(name="singles", bufs=1))
        xpool = ctx.enter_context(tc.tile_pool(name="xpool", bufs=20))
        ppool = ctx.enter_context(tc.tile_pool(name="ppool", bufs=8))
        proj_ps = ctx.enter_context(tc.tile_pool(name="proj_ps", bufs=2, space="PSUM"))
        sc_ps = ctx.enter_context(tc.tile_pool(name="sc_ps", bufs=2, space="PSUM"))
        pv_ps = ctx.enter_context(tc.tile_pool(name="pv_ps", bufs=2, space="PSUM"))
        sum_ps = ctx.enter_context(tc.tile_pool(name="sum_ps", bufs=2, space="PSUM"))

        # ---- constants (ACT HWDGE ring; xt stream owns the SP ring) ----
        w_sb = {}
        for name, dram in (("wk", wk_d), ("wv", wv_d), ("wq", wq_d)):
            t = singles.tile([P, NEB, D], bf16, tag=f"w_{name}")
            nc.scalar.dma_start(t[:], dram.rearrange("(o p) d -> p o d", p=P))
            w_sb[name] = t
        b_sb = {}
        for name, dram in (("bq", bq_d), ("bk", bk_d), ("bv", bv_d)):
            t = singles.tile([P, 1], f32, tag=f"b_{name}")
            nc.scalar.dma_start(t[:], dram.rearrange("(p one) -> p one", one=1))
            b_sb[name] = t
        mask2 = singles.tile([P, 2 * P], bf16, tag="mask2")
        nc.scalar.dma_start(mask2[:], mask_d[:])
        identb = singles.tile([P, P], bf16, tag="identb")
        nc.scalar.dma_start(identb[:], identb_d[:])
        onesc = singles.tile([P, 1], bf16, tag="onesc")
        nc.scalar.dma_start(onesc[:], onesc_d[:])

        # ---- persistent activations ----
        kt = singles.tile([P, LT, P], bf16, tag="kt")      # K^T  [d, lt, k]
        vt = singles.tile([P, LT, P], bf16, tag="vt")      # V^T  [d, lt, s]
        vn = singles.tile([P, LT, D], bf16, tag="vn")      # V natural [s, lt, d]
        qt = singles.tile([P, NT, P], bf16, tag="qt")      # Q^T [d, storage p, q]
        pvt_sb = singles.tile([D, S], f32, tag="pvt_sb")
        sums_sb = singles.tile([1, S], f32, tag="sums_sb")
        qt4 = qt.rearrange("p (j two) k -> p j two k", two=2)

        def proj_kv_blk(sb):
            """K/V/Q projections for kv-half s-block sb (512 cols).

            Q result lands at even storage positions 8sb, 8sb+2, ..."""
            xts = []
            for eo in range(NEB):
                xtile = xpool.tile([P, QBW], bf16, tag="xt")
                eng = nc.sync
                eng.dma_start(
                    xtile[:], xt_kv[eo * P : (eo + 1) * P, sb * QBW : (sb + 1) * QBW]
                )
                xts.append(xtile)
            # K first (feeds scores), split copy so attention starts on
            # the first half-tile-pair early; then Q (same), V last (only
            # needed post-exp).
            ps = proj_ps.tile([P, QBW], f32, tag="ps")
            for eo in range(NEB):
                nc.tensor.matmul(
                    ps[:], w_sb["wk"][:, eo, :], xts[eo][:],
                    start=(eo == 0), stop=(eo == NEB - 1),
                )
            ktv = kt.rearrange("p lt k -> p (lt k)")
            for half in range(2):
                sl = slice(sb * QBW + half * 256, sb * QBW + half * 256 + 256)
                nc.vector.tensor_scalar_add(
                    ktv[:, sl], ps[:, half * 256 : half * 256 + 256],
                    b_sb["bk"][:],
                )
            ps = proj_ps.tile([P, QBW], f32, tag="ps")
            for eo in range(NEB):
                nc.tensor.matmul(
                    ps[:], w_sb["wq"][:, eo, :], xts[eo][:],
                    start=(eo == 0), stop=(eo == NEB - 1),
                )
            psj = ps.rearrange("p (j k) -> p j k", k=P)
            for half in range(2):
                nc.vector.tensor_scalar(
                    qt4[:, 4 * sb + 2 * half : 4 * sb + 2 * half + 2, 0, :],
                    psj[:, 2 * half : 2 * half + 2, :],
                    SCALE,
                    b_sb["bq"][:],
                    mybir.AluOpType.mult,
                    mybir.AluOpType.add,
                )
            ps = proj_ps.tile([P, QBW], f32, tag="ps")
            for eo in range(NEB):
                nc.tensor.matmul(
                    ps[:], w_sb["wv"][:, eo, :], xts[eo][:],
                    start=(eo == 0), stop=(eo == NEB - 1),
                )
            vtv = vt.rearrange("p lt k -> p (lt k)")
            nc.vector.tensor_scalar_add(
                vtv[:, sb * QBW : (sb + 1) * QBW], ps[:], b_sb["bv"][:]
            )

        def proj_q_oth(sb):
            """Q projection for oth-local s-tiles 4sb..4sb+3 (odd storage)."""
            xts = []
            for eo in range(NEB):
                xtile = xpool.tile([P, QBW], bf16, tag="xt")
                eng = nc.sync
                eng.dma_start(
                    xtile[:],
                    xt_oth[eo * P : (eo + 1) * P, sb * QBW : (sb + 1) * QBW],
                )
                xts.append(xtile)
            ps = proj_ps.tile([P, QBW], f32, tag="ps")
            for eo in range(NEB):
                nc.tensor.matmul(
                    ps[:],
                    w_sb["wq"][:, eo, :],
                    xts[eo][:],
                    start=(eo == 0),
                    stop=(eo == NEB - 1),
                )
            psj = ps.rearrange("p (j k) -> p j k", k=P)
            for half in range(2):
                nc.vector.tensor_scalar(
                    qt4[:, 4 * sb + 2 * half : 4 * sb + 2 * half + 2, 1, :],
                    psj[:, 2 * half : 2 * half + 2, :],
                    SCALE,
                    b_sb["bq"][:],
                    mybir.AluOpType.mult,
                    mybir.AluOpType.add,
                )

        def v_transpose(lt):
            ps = sc_ps.tile([P, P], bf16, tag="sc")
            nc.tensor.transpose(ps[:], vt[:, lt, :], identb[:])
            nc.vector.tensor_copy(out=vn[:, lt, :], in_=ps[:, :D])

        def attention_blk(b):
            """Attention for q-block b = storage positions 4b..4b+3."""
            qview = qt[:, 4 * b : 4 * b + 4, :]               # [d, 4, 128] = 512 q
            col0 = b * QBW
            pv = pv_ps.tile([P, QBW], f32, tag="pv")
            sm = sum_ps.tile([1, QBW], f32, tag="sm")
            for i in range(2 * b + 2):
                sc = sc_ps.tile([P, QBW], f32, tag="sc")
                first = i == 0
                last = i == 2 * b + 1
                if i < 2 * b:
                    c0 = 0
                    nc.tensor.matmul(sc[:], kt[:, i, :], qview, start=True, stop=True)
                else:
                    # in-block k-tile: mask segment [c0, c0+256) + clean tail
                    c0 = 256 * (i - 2 * b)
                    nc.tensor.matmul(
                        sc[:, c0 : c0 + 256],
                        identb[:],
                        mask2[:],
                        start=True,
                        stop=False,
                    )
                    nc.tensor.matmul(
                        sc[:, c0 : c0 + 256],
                        kt[:, i, :],
                        qview.rearrange("p j k -> p (j k)")[:, c0 : c0 + 256],
                        start=False,
                        stop=True,
                    )
                    if c0 == 0:
                        nc.tensor.matmul(
                            sc[:, 256:],
                            kt[:, i, :],
                            qview.rearrange("p j k -> p (j k)")[:, 256:],
                            start=True,
                            stop=True,
                        )
                w = QBW - c0
                p = ppool.tile([P, QBW], bf16, tag="p")
                nc.scalar.activation(
                    p[:, :w], sc[:, c0:], mybir.ActivationFunctionType.Exp
                )
                nc.tensor.matmul(
                    pv[:, c0:], vn[:, i, :], p[:, :w], start=first, stop=last
                )
                nc.tensor.matmul(
                    sm[:, c0:], onesc[:], p[:, :w], start=first, stop=last
                )
            nc.vector.tensor_copy(out=pvt_sb[:, col0 : col0 + QBW], in_=pv[:])
            nc.vector.tensor_copy(out=sums_sb[:, col0 : col0 + QBW], in_=sm[:])
            out_eng = nc.gpsimd if b < 3 else nc.sync
            out_eng.dma_start(
                pvt_d[:, col0 : col0 + QBW], pvt_sb[:, col0 : col0 + QBW]
            )
            out_eng.dma_start(
                sums_d[:, col0 : col0 + QBW], sums_sb[:, col0 : col0 + QBW]
            )

        # ---- emission order (priority hint for the scheduler) ----
        for _rep in range(reps):
            proj_kv_blk(0)
            for lt in range(4):
                v_transpose(lt)
            proj_q_oth(0)
            attention_blk(0)
            attention_blk(1)
            proj_kv_blk(1)
            for lt in range(4, LT):
                v_transpose(lt)
            proj_q_oth(1)
            attention_blk(2)
            attention_blk(3)

    nc.compile()
    return nc


def _get_module(reps=1):
    key = ("nc", reps)
    if key not in _CACHE:
        _CACHE[key] = _build_module(reps)
    return _CACHE[key]


def _host_prep(x, Wq, bq, Wk, bk, Wv, bv):
    """Build the 8 per-core input maps plus per-core q-column permutations."""
    x = np.asarray(x, dtype=np.float32)
    bf = ml_dtypes.bfloat16
    in_maps = []
    perms = []
    # triangle: key ki (partition), query qi (col): visible iff qi >= ki
    ki = np.arange(P)[:, None]
    qi = np.arange(P)[None, :]
    tri = np.where(qi >= ki, 0.0, NEG).astype(np.float32)
    for c in range(8):
        b, h = divmod(c, 2)
        xt = np.ascontiguousarray(x[b].T)             # [E, S]
        xt3 = xt.reshape(E, NT, P)
        xt_kv = np.ascontiguousarray(xt3[:, h::2, :].reshape(E, S // 2)).astype(bf)
        xt_oth = np.ascontiguousarray(
            xt3[:, 1 - h :: 2, :].reshape(E, S // 2)
        ).astype(bf)
        # mask segment for in-block k-tile at suffix offset c0: cols [0,128)
        # = storage p (global p^h): h=0 diag-tri, h=1 k-tile is global+1 ->
        # tri; cols [128,256) = storage p+1: h=0 future-unmasked (0),
        # h=1 global p -> fully masked.
        seg = np.concatenate(
            [tri, np.full((P, P), NEG if h == 1 else 0.0, np.float32)], axis=1
        )
        in_maps.append(
            {
                "xt_kv": xt_kv,
                "xt_oth": xt_oth,
                "wq": np.asarray(Wq, np.float32).astype(bf),
                "wk": np.asarray(Wk, np.float32).astype(bf),
                "wv": np.asarray(Wv, np.float32).astype(bf),
                "bq": np.asarray(bq, np.float32) * np.float32(SCALE),
                "bk": np.asarray(bk, np.float32),
                "bv": np.asarray(bv, np.float32),
                "mask2": np.ascontiguousarray(seg.astype(bf)),
                "identb": np.eye(P, dtype=bf),
                "onesc": np.ones((P, 1), dtype=bf),
            }
        )
        # storage position p holds global q-tile g = p ^ h
        perm = np.empty(S, dtype=np.int64)
        for p_ in range(NT):
            g = p_ ^ h
            perm[p_ * P : (p_ + 1) * P] = g * P + np.arange(P)
        perms.append(perm)
    return in_maps, perms


def kernel(x, Wq, bq, Wk, bk, Wv, bv):
    from concourse.bass_utils import run_bass_kernel_spmd

    nc = _get_module()
    in_maps, perms = _host_prep(x, Wq, bq, Wk, bk, Wv, bv)
    res = run_bass_kernel_spmd(
        nc,
        in_maps,
        core_ids=list(range(8)),
        trace=TRACE,
        **TRACE_KW,
    )
    _CACHE["last_result"] = res

    out = np.empty((B, S, D), dtype=np.float32)
    for b in range(B):
        r0, r1 = res.results[2 * b], res.results[2 * b + 1]
        pv = np.zeros((D, S), dtype=np.float64)
        sm = np.zeros((S,), dtype=np.float64)
        for r, perm in ((r0, perms[2 * b]), (r1, perms[2 * b + 1])):
            pv[:, perm] += r["pvt"].astype(np.float64)
            sm[perm] += r["sums"][0].astype(np.float64)
        out[b] = (pv / sm[None, :]).T.astype(np.float32)
    return out


# revision 31
# speedup vs baseline: 1.0092x; 1.0092x over previous
"""Causal self-attention (B=4, S=2048, E=1024, D=128, single head) on 8 TRN2 cores.

Sharding: core c = 2*b + h handles batch b; the two cores of a pair split the
causal key range by k-tile parity (h=0 even 128-row k-tiles, h=1 odd).  All 8
cores run the *same* instruction stream; per-core differences live in DRAM
data (host-permuted x columns and a parity-dependent mask tile).

Query storage order: position p <-> global q-tile g = p ^ h.  This makes the
causal structure suffix-contiguous and parity-independent: attention block b
covers storage positions 4b..4b+3 (= global tiles {4b..4b+3} for both
parities); its own-parity k-tiles are exactly i = 0..2b+1, where k-tile i is
"in-block" iff i >= 2b.  Out-of-block k-tiles (i < 2b) need no mask and a full
512-wide scores stream; the two in-block k-tiles (i = 2b, 2b+1) stream only
the causal suffix (cols 0/256..512) with a single 256-wide host mask segment
[tri | 0 or -inf] pre-loaded into PSUM via an identity matmul (start=False
accumulation keeps masking off the DVE/ACT chain).  vs. the parity-blocked
predecessor this removes all rank-1 code masks and ~25% of scores/pv/sums
matmul columns.

All matmuls run in bf16 (fp32 PSUM accumulation).  K/V projections cover the
own-parity half; Q covers all 2048 queries (kv-parity tiles land at even
storage p via a strided write, oth-parity at odd p).  pv/sum/proj PSUM pools
are double-buffered so copy-outs overlap the next block's matmuls.
Each core emits unnormalized PV partials (pvT [128 d, 2048 q]) and softmax
denominators (sums [1, 2048]); the host combines the pair:
  out[b] = ((pv0 + pv1) / (s0 + s1)).T  (+ per-core q-column de-permutation)
"""

import os

os.environ.setdefault("MYCRO_LOCAL_CACHE", "1")

import ml_dtypes
import numpy as np

B, S, E, D = 4, 2048, 1024, 128
P = 128
NT = S // P          # 16 global k-tiles per batch
LT = NT // 2         # 8 local (per-core) k-tiles
NQB = 4              # 512-wide query blocks
QBW = 512
NEB = E // P         # 8 e-tiles
SCALE = 1.0 / float(np.sqrt(D))
NEG = -1.0e30

TRACE = False        # set by test.py for profiling runs
TRACE_KW = {}

_CACHE = {}


def _build_module(reps=1):
    from contextlib import ExitStack

    import concourse.bacc as bacc
    import concourse.mybir as mybir
    import concourse.tile as tile

    f32 = mybir.dt.float32
    bf16 = mybir.dt.bfloat16

    nc = bacc.Bacc("TRN2", target_bir_lowering=False, debug=False, num_devices=8)

    xt_kv = nc.dram_tensor("xt_kv", [P, NEB * (S // 2)], bf16, kind="ExternalInput").ap()
    xt_oth = nc.dram_tensor("xt_oth", [P, NEB * (S // 2)], bf16, kind="ExternalInput").ap()
    wq_d = nc.dram_tensor("wq", [E, D], bf16, kind="ExternalInput").ap()
    wk_d = nc.dram_tensor("wk", [E, D], bf16, kind="ExternalInput").ap()
    wv_d = nc.dram_tensor("wv", [E, D], bf16, kind="ExternalInput").ap()
    bq_d = nc.dram_tensor("bq", [D], f32, kind="ExternalInput").ap()  # pre-scaled
    bk_d = nc.dram_tensor("bk", [D], f32, kind="ExternalInput").ap()
    bv_d = nc.dram_tensor("bv", [D], f32, kind="ExternalInput").ap()
    # [k=128, 2*128 q] = [tri | h==0 ? 0 : -inf]
    mask_d = nc.dram_tensor("mask2", [P, 2 * P], bf16, kind="ExternalInput").ap()
    identb_d = nc.dram_tensor("identb", [P, P], bf16, kind="ExternalInput").ap()
    onesc_d = nc.dram_tensor("onesc", [P, 1], bf16, kind="ExternalInput").ap()
    pvt_d = nc.dram_tensor("pvt", [D, S], bf16, kind="ExternalOutput").ap()
    sums_d = nc.dram_tensor("sums", [1, S], bf16, kind="ExternalOutput").ap()

    with tile.TileContext(nc) as tc, ExitStack() as ctx:
        singles = ctx.enter_context(tc.tile_pool(name="singles", bufs=1))
        xpool = ctx.enter_context(tc.tile_pool(name="xpool", bufs=2))
        ppool = ctx.enter_context(tc.tile_pool(name="ppool", bufs=8))
        actpool = ctx.enter_context(tc.tile_pool(name="actpool", bufs=2))
        proj_ps = ctx.enter_context(tc.tile_pool(name="proj_ps", bufs=2, space="PSUM"))
        sc_ps = ctx.enter_context(tc.tile_pool(name="sc_ps", bufs=2, space="PSUM"))
        pv_ps = ctx.enter_context(tc.tile_pool(name="pv_ps", bufs=2, space="PSUM"))
        sum_ps = ctx.enter_context(tc.tile_pool(name="sum_ps", bufs=2, space="PSUM"))

        # ---- constants (ACT HWDGE ring; xt stream owns the SP ring) ----
        w_sb = {}
        for name, dram in (("wk", wk_d), ("wv", wv_d), ("wq", wq_d)):
            t = singles.tile([P, NEB, D], bf16, tag=f"w_{name}")
            nc.scalar.dma_start(t[:], dram.rearrange("(o p) d -> p o d", p=P))
            w_sb[name] = t
        b_sb = {}
        for name, dram in (("bq", bq_d), ("bk", bk_d), ("bv", bv_d)):
            t = singles.tile([P, 1], f32, tag=f"b_{name}")
            nc.scalar.dma_start(t[:], dram.rearrange("(p one) -> p one", one=1))
            b_sb[name] = t
        mask2 = singles.tile([P, 2 * P], bf16, tag="mask2")
        nc.scalar.dma_start(mask2[:], mask_d[:])
        identb = singles.tile([P, P], bf16, tag="identb")
        nc.scalar.dma_start(identb[:], identb_d[:])
        onesc = singles.tile([P, 1], bf16, tag="onesc")
        nc.scalar.dma_start(onesc[:], onesc_d[:])

        # ---- per-rep activations (double-buffered so the next rep's
        # projections overlap this rep's attention tail) ----
        pvt_sb = singles.tile([D, S], bf16, tag="pvt_sb")
        sums_sb = singles.tile([1, S], bf16, tag="sums_sb")
        kt = vt = vn = qt = qt4 = None

        def fetch_x(dram, tag):
            # partition-major DRAM: one 8KB-contiguous line per partition
            # per DMA -> 128 descriptors instead of 128 per 512-col tile
            xt = xpool.tile([P, NEB, S // 2], bf16, tag=tag)
            half = NEB * (S // 2) // 2
            xv = xt.rearrange("p e s -> p (e s)")
            nc.sync.dma_start(xv[:, :half], dram[:, :half])
            nc.sync.dma_start(xv[:, half:], dram[:, half:])
            return xt

        def proj_kv_blk(sb, xkv):
            """K/V/Q projections for kv-half s-block sb (512 cols).

            Q result lands at even storage positions 8sb, 8sb+2, ..."""
            xts = [
                xkv[:, eo, sb * QBW : (sb + 1) * QBW] for eo in range(NEB)
            ]
            # K first (feeds scores), split copy so attention starts on
            # the first half-tile-pair early; then Q (same), V last (only
            # needed post-exp).
            ps = proj_ps.tile([P, QBW], f32, tag="ps")
            for eo in range(NEB):
                nc.tensor.matmul(
                    ps[:], w_sb["wk"][:, eo, :], xts[eo],
                    start=(eo == 0), stop=(eo == NEB - 1),
                )
            ktv = kt.rearrange("p lt k -> p (lt k)")
            for half in range(2):
                sl = slice(sb * QBW + half * 256, sb * QBW + half * 256 + 256)
                nc.vector.tensor_scalar_add(
                    ktv[:, sl], ps[:, half * 256 : half * 256 + 256],
                    b_sb["bk"][:],
                )
            ps = proj_ps.tile([P, QBW], f32, tag="ps")
            for eo in range(NEB):
                nc.tensor.matmul(
                    ps[:], w_sb["wq"][:, eo, :], xts[eo],
                    start=(eo == 0), stop=(eo == NEB - 1),
                )
            psj = ps.rearrange("p (j k) -> p j k", k=P)
            for half in range(2):
                nc.vector.tensor_scalar(
                    qt4[:, 4 * sb + 2 * half : 4 * sb + 2 * half + 2, 0, :],
                    psj[:, 2 * half : 2 * half + 2, :],
                    SCALE,
                    b_sb["bq"][:],
                    mybir.AluOpType.mult,
                    mybir.AluOpType.add,
                )
            ps = proj_ps.tile([P, QBW], f32, tag="ps")
            for eo in range(NEB):
                nc.tensor.matmul(
                    ps[:], w_sb["wv"][:, eo, :], xts[eo],
                    start=(eo == 0), stop=(eo == NEB - 1),
                )
            vtv = vt.rearrange("p lt k -> p (lt k)")
            nc.vector.tensor_scalar_add(
                vtv[:, sb * QBW : (sb + 1) * QBW], ps[:], b_sb["bv"][:]
            )

        def proj_q_oth(sb, xoth):
            """Q projection for oth-local s-tiles 4sb..4sb+3 (odd storage)."""
            ps = proj_ps.tile([P, QBW], f32, tag="ps")
            for eo in range(NEB):
                nc.tensor.matmul(
                    ps[:],
                    w_sb["wq"][:, eo, :],
                    xoth[:, eo, sb * QBW : (sb + 1) * QBW],
                    start=(eo == 0),
                    stop=(eo == NEB - 1),
                )
            psj = ps.rearrange("p (j k) -> p j k", k=P)
            for half in range(2):
                nc.vector.tensor_scalar(
                    qt4[:, 4 * sb + 2 * half : 4 * sb + 2 * half + 2, 1, :],
                    psj[:, 2 * half : 2 * half + 2, :],
                    SCALE,
                    b_sb["bq"][:],
                    mybir.AluOpType.mult,
                    mybir.AluOpType.add,
                )

        def v_transpose(lt):
            ps = sc_ps.tile([P, P], bf16, tag="sc")
            nc.tensor.transpose(ps[:], vt[:, lt, :], identb[:])
            nc.vector.tensor_copy(out=vn[:, lt, :], in_=ps[:, :D])

        def attn_ktile(b, i, pv, sm):
            """Block b (storage 4b..4b+3), own-parity k-tile i (0..2b+1)."""
            qview = qt[:, 4 * b : 4 * b + 4, :]
            sc = sc_ps.tile([P, QBW], f32, tag="sc")
            first = i == 0
            last = i == 2 * b + 1
            if i < 2 * b:
                c0 = 0
                nc.tensor.matmul(sc[:], kt[:, i, :], qview, start=True, stop=True)
            else:
                # in-block k-tile: mask segment [c0, c0+256) + clean tail
                c0 = 256 * (i - 2 * b)
                nc.tensor.matmul(
                    sc[:, c0 : c0 + 256],
                    identb[:],
                    mask2[:],
                    start=True,
                    stop=False,
                )
                nc.tensor.matmul(
                    sc[:, c0 : c0 + 256],
                    kt[:, i, :],
                    qview.rearrange("p j k -> p (j k)")[:, c0 : c0 + 256],
                    start=False,
                    stop=True,
                )
                if c0 == 0:
                    nc.tensor.matmul(
                        sc[:, 256:],
                        kt[:, i, :],
                        qview.rearrange("p j k -> p (j k)")[:, 256:],
                        start=True,
                        stop=True,
                    )
            w = QBW - c0
            p = ppool.tile([P, QBW], bf16, tag="p")
            nc.scalar.activation(
                p[:, :w], sc[:, c0:], mybir.ActivationFunctionType.Exp
            )
            nc.tensor.matmul(
                pv[:, c0:], vn[:, i, :], p[:, :w], start=first, stop=last
            )
            nc.tensor.matmul(
                sm[:, c0:], onesc[:], p[:, :w], start=first, stop=last
            )

        def attn_drain(b, pv, sm):
            col0 = b * QBW
            nc.vector.tensor_copy(out=pvt_sb[:, col0 : col0 + QBW], in_=pv[:])
            nc.vector.tensor_copy(out=sums_sb[:, col0 : col0 + QBW], in_=sm[:])

        def attention_blk(b):
            pv = pv_ps.tile([P, QBW], f32, tag="pv")
            sm = sum_ps.tile([1, QBW], f32, tag="sm")
            for i in range(2 * b + 2):
                attn_ktile(b, i, pv, sm)
            attn_drain(b, pv, sm)

        # ---- emission order (priority hint for the scheduler) ----
        pre_kv = None
        for _rep in range(reps):
            kt = actpool.tile([P, LT, P], bf16, tag="kt")   # K^T [d, lt, k]
            vt = actpool.tile([P, LT, P], bf16, tag="vt")   # V^T [d, lt, s]
            vn = actpool.tile([P, LT, D], bf16, tag="vn")   # V natural
            qt = actpool.tile([P, NT, P], bf16, tag="qt")   # Q^T [d, p, q]
            qt4 = qt.rearrange("p (j two) k -> p j two k", two=2)
            xkv = pre_kv if pre_kv is not None else fetch_x(xt_kv, "xkv")
            xoth = fetch_x(xt_oth, "xoth")
            proj_kv_blk(0, xkv)
            for lt in range(4):
                v_transpose(lt)
            proj_q_oth(0, xoth)
            attention_blk(0)
            attention_blk(1)
            proj_kv_blk(1, xkv)
            for lt in range(4, LT):
                v_transpose(lt)
            proj_q_oth(1, xoth)
            # prefetch next rep's kv x so its projection matmuls are
            # ready fillers during this rep's attention tail
            pre_kv = fetch_x(xt_kv, "xkv")
            attention_blk(2)
            attention_blk(3)
            nc.sync.dma_start(pvt_d[:], pvt_sb[:])
            nc.sync.dma_start(sums_d[:], sums_sb[:])

    nc.compile()
    return nc


def _get_module(reps=1):
    key = ("nc", reps)
    if key not in _CACHE:
        _CACHE[key] = _build_module(reps)
    return _CACHE[key]


def _host_prep(x, Wq, bq, Wk, bk, Wv, bv):
    """Build the 8 per-core input maps plus per-core q-column permutations."""
    x = np.asarray(x, dtype=np.float32)
    bf = ml_dtypes.bfloat16
    in_maps = []
    perms = []
    # triangle: key ki (partition), query qi (col): visible iff qi >= ki
    ki = np.arange(P)[:, None]
    qi = np.arange(P)[None, :]
    tri = np.where(qi >= ki, 0.0, NEG).astype(np.float32)
    for c in range(8):
        b, h = divmod(c, 2)
        xt = np.ascontiguousarray(x[b].T)             # [E, S]
        xt3 = xt.reshape(E, NT, P)
        def part_major(a):
            # [E, S/2] -> [128, NEB*(S/2)]: per partition one contiguous
            # line holding its row of every e-tile
            return np.ascontiguousarray(
                a.reshape(NEB, P, S // 2).transpose(1, 0, 2).reshape(P, -1)
            ).astype(bf)

        xt_kv = part_major(xt3[:, h::2, :].reshape(E, S // 2))
        xt_oth = part_major(xt3[:, 1 - h :: 2, :].reshape(E, S // 2))
        # mask segment for in-block k-tile at suffix offset c0: cols [0,128)
        # = storage p (global p^h): h=0 diag-tri, h=1 k-tile is global+1 ->
        # tri; cols [128,256) = storage p+1: h=0 future-unmasked (0),
        # h=1 global p -> fully masked.
        seg = np.concatenate(
            [tri, np.full((P, P), NEG if h == 1 else 0.0, np.float32)], axis=1
        )
        in_maps.append(
            {
                "xt_kv": xt_kv,
                "xt_oth": xt_oth,
                "wq": np.asarray(Wq, np.float32).astype(bf),
                "wk": np.asarray(Wk, np.float32).astype(bf),
                "wv": np.asarray(Wv, np.float32).astype(bf),
                "bq": np.asarray(bq, np.float32) * np.float32(SCALE),
                "bk": np.asarray(bk, np.float32),
                "bv": np.asarray(bv, np.float32),
                "mask2": np.ascontiguousarray(seg.astype(bf)),
                "identb": np.eye(P, dtype=bf),
                "onesc": np.ones((P, 1), dtype=bf),
            }
        )
        # storage position p holds global q-tile g = p ^ h
        perm = np.empty(S, dtype=np.int64)
        for p_ in range(NT):
            g = p_ ^ h
            perm[p_ * P : (p_ + 1) * P] = g * P + np.arange(P)
        perms.append(perm)
    return in_maps, perms


def kernel(x, Wq, bq, Wk, bk, Wv, bv):
    from concourse.bass_utils import run_bass_kernel_spmd

    nc = _get_module()
    in_maps, perms = _host_prep(x, Wq, bq, Wk, bk, Wv, bv)
    res = run_bass_kernel_spmd(
        nc,
        in_maps,
        core_ids=list(range(8)),
        trace=TRACE,
        **TRACE_KW,
    )
    _CACHE["last_result"] = res

    out = np.empty((B, S, D), dtype=np.float32)
    for b in range(B):
        r0, r1 = res.results[2 * b], res.results[2 * b + 1]
        pv = np.zeros((D, S), dtype=np.float64)
        sm = np.zeros((S,), dtype=np.float64)
        for r, perm in ((r0, perms[2 * b]), (r1, perms[2 * b + 1])):
            pv[:, perm] += r["pvt"].astype(np.float64)
            sm[perm] += r["sums"][0].astype(np.float64)
        out[b] = (pv / sm[None, :]).T.astype(np.float32)
    return out
